# revision 14
# baseline (speedup 1.0000x reference)
"""Multi-head attention (B=2, L=S=2048, D=1024, H=16, E=64) on 8 TRN2 cores.

Sharding: tensor-parallel over heads. Core c owns heads 2c, 2c+1, i.e. the
128-wide slice [c*128:(c+1)*128] of the QKV projection outputs and the
matching row-slice of Wo. Each core reads the full (host-pre-transposed)
queries/keys/values, computes its two heads' attention, and writes a full
[1024, 4096] fp32 partial of the output projection; the host sums the 8
partials, transposes back and adds bo.

On-chip layout (per core):
  XT   = X^T            [1024 dmodel, 4096 tok]   bf16 (DMA'd per k-tile)
  QT/KT = (XW)^T        [128 e', 4096 tok]        fp32 in SBUF
  V'_h = [V_h | 1]      [4096 s, 65]              bf16 (PE-transposed VT)
  scores^T (per s-tile) [128 s, 2*512 (h,l)]      fp32 PSUM (fp32r matmuls)
  P^T = exp(s/8)        [128 s, 2*512]            bf16 SBUF (ScalarE)
  PV: V'_h.T @ P^T_h -> [65, 512] PSUM accumulated over 16 s-tiles;
      row 64 is the softmax denominator (ones column).
  out-proj: Wo_c.T @ OT [128 d, 512 tok] fp32r -> DMA straight to DRAM.
"""

import numpy as np
import ml_dtypes

import concourse.bass as bass
import concourse.bacc as bacc
import concourse.mybir as mybir
from concourse.tile import TileContext
from concourse.bass_utils import run_bass_kernel_spmd

BF16 = mybir.dt.bfloat16
F32 = mybir.dt.float32
F32R = mybir.dt.float32r

B, L, D = 2, 2048, 1024
TOK = B * L              # 4096
H, E = 16, 64
NCORES = 8
E2 = 128                 # projection output dims per core (2 heads)
NKT = D // 128           # 8 k-tiles of the contraction
LQ = 512                 # l-quarter: query-token tile inside attention
NLQ = L // LQ            # 4 per batch
NST = L // 128           # 16 s-tiles (key tokens) per batch
HEADS = 2                # heads per core

_CACHED_NC = None
_IDENT = np.eye(128, dtype=ml_dtypes.bfloat16)


def _f32r(ap):
    return ap.bitcast(F32R)


def build_nc():
    nc = bacc.Bacc("TRN2", target_bir_lowering=False)

    xt = {n: nc.declare_dram_parameter(f"x{n}_t", [D, TOK], BF16, isOutput=False)
          for n in ("q", "k", "v")}
    w = {n: nc.declare_dram_parameter(f"w{n}", [D, E2], BF16, isOutput=False)
         for n in ("q", "k", "v")}
    bias = {n: nc.declare_dram_parameter(f"b{n}", [E2, 1], F32, isOutput=False)
            for n in ("q", "k", "v")}
    wo = nc.declare_dram_parameter("wo", [E2, D], F32, isOutput=False)
    ident_in = nc.declare_dram_parameter("ident_in", [128, 128], BF16, isOutput=False)
    out_t = nc.declare_dram_parameter("out_t", [D, TOK], F32, isOutput=True)
    dn_scratch = nc.dram_tensor("dn_scratch", [B * NLQ * HEADS, LQ], F32)

    with TileContext(nc) as tc:
        with (
            tc.tile_pool(name="const", bufs=1) as const,
            tc.tile_pool(name="persist", bufs=1) as persist,
            tc.tile_pool(name="xt_pool", bufs=10) as xt_pool,
            tc.tile_pool(name="pt_pool", bufs=4) as pt_pool,
            tc.tile_pool(name="norm_pool", bufs=4) as norm_pool,
            tc.tile_pool(name="out_pool", bufs=4) as out_pool,
            tc.tile_pool(name="ps", bufs=3, space="PSUM") as ps,
            tc.tile_pool(name="pv_ps", bufs=2, space="PSUM") as pv_ps,
        ):
            # ---- constants / persistent tensors ----
            ident = const.tile([128, 128], BF16, tag="ident")
            nc.sync.dma_start(out=ident[:], in_=ident_in.ap())

            w_sb = {}
            b_sb = {}
            for n in ("q", "k", "v"):
                w_sb[n] = const.tile([128, NKT * E2], BF16, tag=f"w_{n}", name=f"w_{n}")
                # DRAM [D, E2] -> SBUF [128 kpart, (kt, e')]
                nc.sync.dma_start(
                    out=w_sb[n][:].rearrange("p (kt e) -> p kt e", kt=NKT),
                    in_=w[n].ap().rearrange("(kt p) e -> p kt e", p=128),
                )
                b_dma = const.tile([E2, 1], F32, tag=f"bdma_{n}", name=f"bdma_{n}")
                nc.sync.dma_start(out=b_dma[:], in_=bias[n].ap())
                b_sb[n] = const.tile([E2, 1], F32, tag=f"b_{n}", name=f"b_{n}")
                nc.vector.tensor_copy(b_sb[n][:], b_dma[:])

            wo_f32 = const.tile([E2, D], F32, tag="wo_f32")
            nc.sync.dma_start(out=wo_f32[:], in_=wo.ap())
            wo_sb = const.tile([E2, D], F32R, tag="wo")
            nc.vector.tensor_copy(wo_sb[:], wo_f32[:])

            qt_sbs = [persist.tile([E2, L], F32R, tag=f"qt{b}", name=f"qt{b}")
                      for b in range(B)]
            kt_sbs = [persist.tile([E2, L], F32R, tag=f"kt{b}", name=f"kt{b}")
                      for b in range(B)]
            vt_sbs = [persist.tile([E2, L], BF16, tag=f"vt{b}", name=f"vt{b}")
                      for b in range(B)]
            # V' per head: [128 spart, (16 stile, 65)] with col 64 == 1.0
            vp_sbs = [[persist.tile([128, NST * 65], BF16, tag=f"vp{b}_{h}",
                                    name=f"vp{b}_{h}")
                       for h in range(HEADS)] for b in range(B)]
            ot_sbs = [persist.tile([E2, L], F32R, tag=f"ot{b}", name=f"ot{b}")
                      for b in range(B)]

            for b in range(B):
                for h in range(HEADS):
                    nc.vector.memset(vp_sbs[b][h][:], 1.0)

            proj_out = {"q": qt_sbs, "k": kt_sbs, "v": vt_sbs}

            def project(n, b):
                """Compute (X W_c)^T for tokens of batch b -> proj_out[n]."""
                t0 = b * L
                xts = []
                for kt in range(NKT):
                    xtile = xt_pool.tile([128, L], BF16, tag="xt")
                    nc.sync.dma_start(
                        out=xtile[:],
                        in_=xt[n].ap()[kt * 128:(kt + 1) * 128, t0:t0 + L],
                    )
                    xts.append(xtile)
                for tp in range(L // 1024):
                    acc = ps.tile([128, 1024], F32, tag="ps")
                    for half in range(2):
                        tt = 2 * tp + half
                        for kt in range(NKT):
                            nc.tensor.matmul(
                                acc[:, half * 512:(half + 1) * 512],
                                lhsT=w_sb[n][:, kt * E2:(kt + 1) * E2],
                                rhs=xts[kt][:, tt * 512:(tt + 1) * 512],
                                start=(kt == 0),
                                stop=(kt == NKT - 1),
                            )
                    for half in range(2):
                        tt = 2 * tp + half
                        dst = proj_out[n][b][:, tt * 512:(tt + 1) * 512]
                        nc.vector.tensor_scalar_add(
                            dst, acc[:, half * 512:(half + 1) * 512], b_sb[n][:]
                        )

            def transpose_v(b):
                """VT [128 e', s] -> V'_h [128 s, (stile, 65)] for batch b."""
                for st in range(NST):
                    tp = ps.tile([128, 128], BF16, tag="ps")
                    nc.tensor.transpose(
                        tp[:], vt_sbs[b][:, st * 128:(st + 1) * 128], ident[:]
                    )
                    for h in range(HEADS):
                        nc.vector.tensor_copy(
                            vp_sbs[b][h][:, st * 65 + 0: st * 65 + 64],
                            tp[:, h * 64:(h + 1) * 64],
                        )

            def attention(b, lq):
                """One (batch, l-quarter): 16 s-tiles -> OT columns."""
                l0 = lq * LQ
                pv = [pv_ps.tile([65, LQ], F32, tag="pv", name=f"pv_{b}_{lq}_{h}") for h in range(HEADS)]
                for st in range(NST):
                    s0 = st * 128
                    sc = ps.tile([128, 2 * LQ], F32, tag="ps")
                    for h in range(HEADS):
                        nc.tensor.matmul(
                            sc[:, h * LQ:(h + 1) * LQ],
                            lhsT=kt_sbs[b][h * 64:(h + 1) * 64, s0:s0 + 128],
                            rhs=qt_sbs[b][h * 64:(h + 1) * 64, l0:l0 + LQ],
                            start=True, stop=True,
                        )
                    pt = pt_pool.tile([128, 2 * LQ], BF16, tag="pt")
                    nc.scalar.activation(
                        pt[:], sc[:], mybir.ActivationFunctionType.Exp,
                        scale=0.125,
                    )
                    for h in range(HEADS):
                        nc.tensor.matmul(
                            pv[h][:],
                            lhsT=vp_sbs[b][h][:, st * 65:(st + 1) * 65],
                            rhs=pt[:, h * LQ:(h + 1) * LQ],
                            start=(st == 0), stop=(st == NST - 1),
                        )
                # normalize: rows 0..63 / row 64
                for h in range(HEADS):
                    rec = norm_pool.tile([1, LQ], F32, tag="rec")
                    nc.vector.reciprocal(rec[:], pv[h][64:65, :])
                    bc = norm_pool.tile([64, LQ], F32, tag="bc")
                    idx = (b * NLQ + lq) * HEADS + h
                    nc.sync.dma_start(
                        out=dn_scratch.ap()[idx:idx + 1, :], in_=rec[:]
                    )
                    nc.sync.dma_start(
                        out=bc[:],
                        in_=dn_scratch.ap()[idx:idx + 1, :].to_broadcast([64, LQ]),
                    )
                    nc.vector.tensor_tensor(
                        out=ot_sbs[b][h * 64:(h + 1) * 64, l0:l0 + LQ],
                        in0=pv[h][0:64, :],
                        in1=bc[:],
                        op=mybir.AluOpType.mult,
                    )

            def out_proj(b):
                t0 = b * L
                for tt in range(L // 512):
                    for dt in range(D // 128):
                        op = ps.tile([128, 512], F32, tag="ps")
                        nc.tensor.matmul(
                            op[:],
                            lhsT=wo_sb[:, dt * 128:(dt + 1) * 128],
                            rhs=ot_sbs[b][:, tt * 512:(tt + 1) * 512],
                            start=True, stop=True,
                        )
                        ob = out_pool.tile([128, 512], F32, tag="ob")
                        nc.vector.tensor_copy(ob[:], op[:])
                        nc.sync.dma_start(
                            out=out_t.ap()[dt * 128:(dt + 1) * 128,
                                           t0 + tt * 512: t0 + (tt + 1) * 512],
                            in_=ob[:],
                        )

            for b in range(B):
                for n in ("q", "k", "v"):
                    project(n, b)
                transpose_v(b)
                for lq in range(NLQ):
                    attention(b, lq)
                out_proj(b)

    nc.compile()
    return nc


def _get_nc():
    global _CACHED_NC
    if _CACHED_NC is None:
        _CACHED_NC = build_nc()
    return _CACHED_NC


def _prep_inputs(queries, keys, values, Wq, bq, Wk, bk, Wv, bv, Wo, bo):
    bf16 = ml_dtypes.bfloat16
    x_t = {}
    for n, arr in (("q", queries), ("k", keys), ("v", values)):
        x_t[n] = np.ascontiguousarray(
            np.asarray(arr, np.float32).reshape(TOK, D).T
        ).astype(bf16)
    in_maps = []
    for c in range(NCORES):
        sl = slice(c * E2, (c + 1) * E2)
        m = {
            "xq_t": x_t["q"], "xk_t": x_t["k"], "xv_t": x_t["v"],
            "wq": np.ascontiguousarray(np.asarray(Wq, np.float32)[:, sl]).astype(bf16),
            "wk": np.ascontiguousarray(np.asarray(Wk, np.float32)[:, sl]).astype(bf16),
            "wv": np.ascontiguousarray(np.asarray(Wv, np.float32)[:, sl]).astype(bf16),
            "bq": np.ascontiguousarray(np.asarray(bq, np.float32)[sl].reshape(E2, 1)),
            "bk": np.ascontiguousarray(np.asarray(bk, np.float32)[sl].reshape(E2, 1)),
            "bv": np.ascontiguousarray(np.asarray(bv, np.float32)[sl].reshape(E2, 1)),
            "wo": np.ascontiguousarray(np.asarray(Wo, np.float32)[sl, :]),
            "ident_in": _IDENT,
        }
        in_maps.append(m)
    return in_maps


def _postprocess(results, bo):
    acc = np.zeros((D, TOK), np.float64)
    for r in results:
        acc += r["out_t"].astype(np.float64)
    out = acc.T.astype(np.float32) + np.asarray(bo, np.float32)[None, :]
    return out.reshape(B, L, D)


def run(trace=False, **inputs):
    nc = _get_nc()
    in_maps = _prep_inputs(**inputs)
    res = run_bass_kernel_spmd(nc, in_maps, core_ids=list(range(NCORES)),
                               trace=trace)
    out = _postprocess(res.results, inputs["bo"])
    return out, res


def kernel(**inputs):
    out, _ = run(trace=False, **inputs)
    return out


# revision 39
# speedup vs baseline: 1.3162x; 1.3162x over previous
"""Multi-head attention (B=2, L=S=2048, D=1024, H=16, E=64) on 8 TRN2 cores.

Sharding: tensor-parallel over heads. Core c owns heads 2c, 2c+1, i.e. the
128-wide slice [c*128:(c+1)*128] of the QKV projection outputs and the
matching row-slice of Wo. Each core reads the full (host-pre-transposed)
queries/keys/values, computes its two heads' attention, and writes a full
[1024, 4096] fp32 partial of the output projection; the host sums the 8
partials, transposes back and adds bo.

On-chip layout (per core):
  XT   = X^T            [1024 dmodel, 4096 tok]   bf16 (DMA'd per k-tile)
  QT/KT = (XW)^T        [128 e', 4096 tok]        fp32 in SBUF
  V'_h = [V_h | 1]      [4096 s, 65]              bf16 (PE-transposed VT)
  scores^T (per s-tile) [128 s, 2*512 (h,l)]      fp32 PSUM (fp32r matmuls)
  P^T = exp(s/8)        [128 s, 2*512]            bf16 SBUF (ScalarE)
  PV: V'_h.T @ P^T_h -> [65, 512] PSUM accumulated over 16 s-tiles;
      row 64 is the softmax denominator (ones column).
  out-proj: Wo_c.T @ OT [128 d, 512 tok] fp32r -> DMA straight to DRAM.
"""

import numpy as np
import ml_dtypes

import concourse.bass as bass
import concourse.bacc as bacc
import concourse.mybir as mybir
from concourse.tile import TileContext
from concourse.bass_utils import run_bass_kernel_spmd

BF16 = mybir.dt.bfloat16
F32 = mybir.dt.float32
F32R = mybir.dt.float32r

B, L, D = 2, 2048, 1024
TOK = B * L              # 4096
H, E = 16, 64
NCORES = 8
E2 = 128                 # projection output dims per core (2 heads)
NKT = D // 128           # 8 k-tiles of the contraction
LQ = 512                 # l-quarter: query-token tile inside attention
NLQ = L // LQ            # 4 per batch
NST = L // 128           # 16 s-tiles (key tokens) per batch
HEADS = 2                # heads per core

_CACHED_NC = None
_IDENT = np.eye(128, dtype=ml_dtypes.bfloat16)


def _warrange(w):
    # [D, E2] -> [128, NKT*E2]: row p holds [w[kt*128+p, :] for kt]
    return np.ascontiguousarray(
        w.reshape(NKT, 128, E2).transpose(1, 0, 2).reshape(128, NKT * E2)
    ).astype(ml_dtypes.bfloat16)


def _f32r(ap):
    return ap.bitcast(F32R)


def build_nc():
    nc = bacc.Bacc("TRN2", target_bir_lowering=False)

    xt = {n: nc.declare_dram_parameter(f"x{n}_t", [D, TOK], BF16, isOutput=False)
          for n in ("q", "k", "v")}
    w = {n: nc.declare_dram_parameter(f"w{n}", [128, NKT * E2], BF16,
                                      isOutput=False)
         for n in ("q", "k", "v")}
    bias = {n: nc.declare_dram_parameter(f"b{n}", [E2, 1], F32, isOutput=False)
            for n in ("q", "k", "v")}
    wo = nc.declare_dram_parameter("wo", [E2, D], F32, isOutput=False)
    ident_in = nc.declare_dram_parameter("ident_in", [128, 128], BF16, isOutput=False)
    out_t = nc.declare_dram_parameter("out_t", [D, TOK], F32, isOutput=True)

    with TileContext(nc) as tc:
        with (
            tc.tile_pool(name="const", bufs=1) as const,
            tc.tile_pool(name="persist", bufs=1) as persist,
            tc.tile_pool(name="xt_pool", bufs=10) as xt_pool,
            tc.tile_pool(name="pt_pool", bufs=16) as pt_pool,
            tc.tile_pool(name="norm_pool", bufs=4) as norm_pool,
            tc.tile_pool(name="out_pool", bufs=4) as out_pool,
            tc.tile_pool(name="sc_ps", bufs=2, space="PSUM") as sc_ps,
            tc.tile_pool(name="misc_ps", bufs=2, space="PSUM") as misc_ps,
            tc.tile_pool(name="pv_ps", bufs=2, space="PSUM") as pv_ps,
        ):
            # ---- constants / persistent tensors (loads emitted JIT below) ----
            ident = const.tile([128, 128], BF16, tag="ident")
            w_sb = {n: const.tile([128, NKT * E2], BF16, tag=f"w_{n}",
                                  name=f"w_{n}") for n in ("q", "k", "v")}
            b_sb = {}
            for n in ("q", "k", "v"):
                b_dma = const.tile([E2, 1], F32, tag=f"bdma_{n}", name=f"bdma_{n}")
                nc.gpsimd.dma_start(out=b_dma[:], in_=bias[n].ap())
                b_sb[n] = const.tile([E2, 1], F32, tag=f"b_{n}", name=f"b_{n}")
                nc.vector.tensor_copy(b_sb[n][:], b_dma[:])

            ones_f = const.tile([1, 64], F32, tag="ones_f")
            nc.vector.memset(ones_f[:], 1.0)
            ones_r = const.tile([1, 64], F32R, tag="ones_r")
            nc.vector.tensor_copy(ones_r[:], ones_f[:])
            warm = const.tile([1, 2], F32, tag="warm")
            nc.vector.memset(warm[:], 0.0)
            nc.scalar.activation(warm[:], warm[:],
                                 mybir.ActivationFunctionType.Exp)
            wo_f32 = const.tile([E2, D], F32, tag="wo_f32")
            wo_sb = const.tile([E2, D], F32R, tag="wo")

            qt_sbs = [[persist.tile([E2, LQ], F32R, tag=f"qt{b}_{t}",
                                    name=f"qt{b}_{t}") for t in range(4)]
                      for b in range(B)]
            kt_sbs = [[persist.tile([E2, LQ], F32R, tag=f"kt{b}_{t}",
                                    name=f"kt{b}_{t}") for t in range(4)]
                      for b in range(B)]
            vt_sbs = [[persist.tile([E2, 512], BF16, tag=f"vt{b}_{g}",
                                     name=f"vt{b}_{g}") for g in range(4)]
                      for b in range(B)]
            # V' per head: [128 spart, (16 stile, 65)] with col 64 == 1.0
            vp_sbs = [[[persist.tile([128, 4 * 65], BF16, tag=f"vp{b}_{h}_{g}",
                                      name=f"vp{b}_{h}_{g}") for g in range(4)]
                       for h in range(HEADS)] for b in range(B)]
            ot_sbs = [persist.tile([E2, L], F32R, tag=f"ot{b}", name=f"ot{b}")
                      for b in range(B)]
            otu_sbs = [[persist.tile([65, L], F32, tag=f"otu{b}_{h}",
                                     name=f"otu{b}_{h}")
                        for h in range(HEADS)] for b in range(B)]

            for b in range(B):
                for h in range(HEADS):
                    for g in range(4):
                        nc.vector.memset(vp_sbs[b][h][g][:], 1.0)

            proj_out = {"q": qt_sbs, "k": kt_sbs, "v": None}

            w_loaded = set()

            def project_loads(n, b):
                t0 = b * L
                if n not in w_loaded:
                    w_loaded.add(n)
                    nc.sync.dma_start(out=w_sb[n][:], in_=w[n].ap())
                xts = []
                for kt in range(NKT):
                    xtile = xt_pool.tile([128, L], BF16, tag="xt")
                    nc.sync.dma_start(
                        out=xtile[:],
                        in_=xt[n].ap()[kt * 128:(kt + 1) * 128, t0:t0 + L],
                    )
                    xts.append(xtile)
                return xts

            def project_mms(n, b, xts):
                for tt in range(L // 512):
                    acc = misc_ps.tile([128, 512], F32, tag="mps")
                    for kt in range(NKT):
                        nc.tensor.matmul(
                            acc[:],
                            lhsT=w_sb[n][:, kt * E2:(kt + 1) * E2],
                            rhs=xts[kt][:, tt * 512:(tt + 1) * 512],
                            start=(kt == 0),
                            stop=(kt == NKT - 1),
                        )
                    if n == "v":
                        dst = vt_sbs[b][tt][:]
                    else:
                        dst = proj_out[n][b][tt][:]
                    nc.vector.tensor_scalar_add(dst, acc[:], b_sb[n][:])

            def project(n, b):
                project_mms(n, b, project_loads(n, b))

            def transpose_v(b):
                """VT [128 e', s] -> V'_h [128 s, (stile, 65)] for batch b."""
                for st in range(NST):
                    g, r = st // 4, st % 4
                    tp = misc_ps.tile([128, 128], BF16, tag="mps")
                    nc.tensor.transpose(
                        tp[:], vt_sbs[b][g][:, r * 128:(r + 1) * 128], ident[:]
                    )
                    for h in range(HEADS):
                        nc.vector.tensor_copy(
                            vp_sbs[b][h][g][:, r * 65: r * 65 + 64],
                            tp[:, h * 64:(h + 1) * 64],
                        )

            def attention_core(b, lq):
                """scores -> exp -> PV for one (batch, l-quarter)."""
                pv = [pv_ps.tile([65, LQ], F32, tag="pv",
                                 name=f"pv_{b}_{lq}_{h}") for h in range(HEADS)]
                qt = qt_sbs[b][lq]
                for st in range(NST):
                    r0 = (st % 4) * 128
                    kt = kt_sbs[b][st // 4]
                    sc = sc_ps.tile([128, 2 * LQ], F32, tag="sc")
                    for h in range(HEADS):
                        nc.tensor.matmul(
                            sc[:, h * LQ:(h + 1) * LQ],
                            lhsT=kt[h * 64:(h + 1) * 64, r0:r0 + 128],
                            rhs=qt[h * 64:(h + 1) * 64, :],
                            start=True, stop=True,
                        )
                    pt = pt_pool.tile([128, 2 * LQ], BF16, tag="pt")
                    nc.scalar.activation(
                        pt[:], sc[:], mybir.ActivationFunctionType.Exp,
                        scale=0.125,
                    )
                    for h in range(HEADS):
                        nc.tensor.matmul(
                            pv[h][:],
                            lhsT=vp_sbs[b][h][st // 4][:, (st % 4) * 65:
                                                       (st % 4) * 65 + 65],
                            rhs=pt[:, h * LQ:(h + 1) * LQ],
                            start=(st == 0), stop=(st == NST - 1),
                        )
                return pv

            def attention_tail(b, lq, pv):
                """pv drain, normalize, out-projection for one unit."""
                l0 = lq * LQ
                for h in range(HEADS):
                    nc.vector.tensor_copy(
                        otu_sbs[b][h][:, l0:l0 + LQ], pv[h][:]
                    )
                for h in range(HEADS):
                    rec = norm_pool.tile([1, LQ], F32R, tag="rec")
                    with nc.allow_low_precision(reason="f32r softmax denom"):
                        nc.vector.reciprocal(
                            rec[:], otu_sbs[b][h][64:65, l0:l0 + LQ]
                        )
                    bcp = misc_ps.tile([64, LQ], F32, tag="mps")
                    nc.tensor.matmul(bcp[:], lhsT=ones_r[:], rhs=rec[:],
                                     start=True, stop=True)
                    nc.vector.tensor_tensor(
                        out=ot_sbs[b][h * 64:(h + 1) * 64, l0:l0 + LQ],
                        in0=otu_sbs[b][h][0:64, l0:l0 + LQ],
                        in1=bcp[:],
                        op=mybir.AluOpType.mult,
                    )
                t0 = b * L
                for dt in range(D // 128):
                    op = misc_ps.tile([128, 512], F32, tag="mps")
                    nc.tensor.matmul(
                        op[:],
                        lhsT=wo_sb[:, dt * 128:(dt + 1) * 128],
                        rhs=ot_sbs[b][:, l0:l0 + LQ],
                        start=True, stop=True,
                    )
                    ob = out_pool.tile([128, 512], F32, tag="ob")
                    nc.vector.tensor_copy(ob[:], op[:])
                    nc.scalar.dma_start(
                        out=out_t.ap()[dt * 128:(dt + 1) * 128,
                                       t0 + l0: t0 + l0 + LQ],
                        in_=ob[:],
                    )

            def load_wo():
                nc.gpsimd.dma_start(out=wo_f32[:], in_=wo.ap())
                nc.vector.tensor_copy(wo_sb[:], wo_f32[:])
                nc.gpsimd.dma_start(out=ident[:], in_=ident_in.ap())

            # schedule: b0 projections; attention units with one-unit-lagged
            # tails; b1 projections interleaved into b0's attention windows.
            project("k", 0)
            project("q", 0)
            load_wo()
            project("v", 0)
            transpose_v(0)
            b1_xts = {n: project_loads(n, 1) for n in ("q", "k", "v")}
            units = [(0, lq) for lq in range(NLQ)] + [(1, lq) for lq in range(NLQ)]
            prev = None
            for i, (b, lq) in enumerate(units):
                pv = attention_core(b, lq)
                if prev is not None:
                    attention_tail(*prev)
                prev = (b, lq, pv)
                if i == 1:
                    project_mms("q", 1, b1_xts["q"])
                elif i == 2:
                    project_mms("k", 1, b1_xts["k"])
                elif i == 3:
                    project_mms("v", 1, b1_xts["v"])
                    transpose_v(1)
            attention_tail(*prev)

    nc.compile()
    return nc


def _get_nc():
    global _CACHED_NC
    if _CACHED_NC is None:
        _CACHED_NC = build_nc()
    return _CACHED_NC


def _prep_inputs(queries, keys, values, Wq, bq, Wk, bk, Wv, bv, Wo, bo):
    bf16 = ml_dtypes.bfloat16
    x_t = {}
    for n, arr in (("q", queries), ("k", keys), ("v", values)):
        x_t[n] = np.ascontiguousarray(
            np.asarray(arr, np.float32).reshape(TOK, D).T
        ).astype(bf16)
    in_maps = []
    for c in range(NCORES):
        sl = slice(c * E2, (c + 1) * E2)
        m = {
            "xq_t": x_t["q"], "xk_t": x_t["k"], "xv_t": x_t["v"],
            "wq": _warrange(np.asarray(Wq, np.float32)[:, sl]),
            "wk": _warrange(np.asarray(Wk, np.float32)[:, sl]),
            "wv": _warrange(np.asarray(Wv, np.float32)[:, sl]),
            "bq": np.ascontiguousarray(np.asarray(bq, np.float32)[sl].reshape(E2, 1)),
            "bk": np.ascontiguousarray(np.asarray(bk, np.float32)[sl].reshape(E2, 1)),
            "bv": np.ascontiguousarray(np.asarray(bv, np.float32)[sl].reshape(E2, 1)),
            "wo": np.ascontiguousarray(np.asarray(Wo, np.float32)[sl, :]),
            "ident_in": _IDENT,
        }
        in_maps.append(m)
    return in_maps


def _postprocess(results, bo):
    acc = np.zeros((D, TOK), np.float64)
    for r in results:
        acc += r["out_t"].astype(np.float64)
    out = acc.T.astype(np.float32) + np.asarray(bo, np.float32)[None, :]
    return out.reshape(B, L, D)


def run(trace=False, **inputs):
    nc = _get_nc()
    in_maps = _prep_inputs(**inputs)
    res = run_bass_kernel_spmd(nc, in_maps, core_ids=list(range(NCORES)),
                               trace=trace)
    out = _postprocess(res.results, inputs["bo"])
    return out, res


def kernel(**inputs):
    out, _ = run(trace=False, **inputs)
    return out
